# revision 9
# baseline (speedup 1.0000x reference)
"""Trainium2 Bass kernel for a GRU decoder with greedy argmax feedback.

Model (13 serial steps, B=64, H=1024, V=32000):
    x      = relu(embedding[tok])
    h      = GRU(x, h)                      # PyTorch gate order r,z,n
    logits = h @ w_out.T + b_out            # [B, V]
    tok    = argmax(logits)                 # feeds next step
Output: log_softmax(logits over all steps)  [B, T, V], and final h.

Sharding over 8 NeuronCores (one TRN2 chip, LNC1):
  - w_out + output logits: vocab-sharded (4000 cols/core, weights resident in SBUF)
  - GRU: gate-dim sharded (each core computes 128 of each of r/z/n) -> AllGather h^T
  - embedding: hidden-dim sharded (each core gathers its 128-wide slice) -> AllGather x^T
  - argmax + log-sum-exp: per-core stats AllGathered, combined identically on all cores

Per step: 3 tiny AllGathers (x^T slices, h^T slices, [max, argmax-idx, sumexp] stats).
All matmuls fp32: the argmax feedback chain cannot tolerate bf16 noise (one flipped
token diverges the whole remaining sequence).
"""

import functools

import ml_dtypes
import numpy as np

B = 64
H = 1024
V = 32000
T = 13
NC = 8
VS = V // NC        # 4000 vocab cols per core
HS = H // NC        # 128 hidden dims per core
KT = H // 128       # 8 K-tiles over hidden
NT = 8              # logits N-tiles
NW = VS // NT       # 500 cols per N-tile (<=512 fp32 moving limit)
PAD = 0
BIG = 1.0e9

RG = [list(range(NC))]


def _build_nc():
    from concourse import bacc, mybir
    import concourse.tile as tile
    from concourse.masks import make_identity
    import concourse.bass as bass

    f32 = mybir.dt.float32
    bf16 = mybir.dt.bfloat16
    i32 = mybir.dt.int32
    u32 = mybir.dt.uint32
    AF = mybir.ActivationFunctionType
    OP = mybir.AluOpType

    nc = bacc.Bacc("TRN2", target_bir_lowering=False, debug=False, num_devices=NC)

    def inp(name, shape, dtype=f32):
        return nc.dram_tensor(name, shape, dtype, kind="ExternalInput")

    def outp(name, shape, dtype=f32):
        return nc.dram_tensor(name, shape, dtype, kind="ExternalOutput")

    w_rz = inp("w_rz", [128, 2 * KT, 256])      # [x|h] K-tiles, cols [r|z]
    w_in = inp("w_in", [128, KT, 128])          # w_ih n-gate slice (transposed)
    w_hn = inp("w_hn", [128, KT, 128])          # w_hh n-gate slice (transposed)
    b_all = inp("b_all", [128, 4])              # cols: -(br), -(bz), 2*b_ih_n, b_hh_n
    w_out = inp("w_out", [128, KT, VS])         # w_out slice, transposed, K-tiled
    b_rep = inp("b_rep", [64, VS], bf16)        # b_out slice replicated over batch
    embT = inp("embT", [V, HS])                 # embedding hidden-slice
    h0T = inp("h0T", [128, KT, 64])             # full h0^T, K-tiled
    h0own = inp("h0own", [128, 64])             # this core's h0^T slice
    vbase = inp("vbase", [64, 1])               # this core's vocab base (c*VS) as f32
    lp = outp("lp", [T, B, VS])
    hfin = outp("hfin", [128, 64])

    with tile.TileContext(nc) as tc:
        with (
            tc.tile_pool(name="sb", bufs=1) as sb,
            tc.tile_pool(name="sbw", bufs=2) as sbw,
            tc.tile_pool(name="sbL", bufs=1) as sbL,
            tc.tile_pool(name="stg", bufs=2) as stg,
            tc.tile_pool(name="psG", bufs=1, space="PSUM") as psG,
            tc.tile_pool(name="psL", bufs=2, space="PSUM") as psL,
            tc.tile_pool(name="psT", bufs=2, space="PSUM") as psT,
            tc.tile_pool(name="dL", bufs=2, space="DRAM") as dL,
            tc.tile_pool(name="dS", bufs=2, space="DRAM") as dS,
        ):
            # ---- static setup ----------------------------------------------
            sw_rz = sb.tile([128, 2 * KT, 256], f32)
            nc.sync.dma_start(out=sw_rz[:], in_=w_rz[:])
            sw_in = sb.tile([128, KT, 128], f32)
            nc.sync.dma_start(out=sw_in[:], in_=w_in[:])
            sw_hn = sb.tile([128, KT, 128], f32)
            nc.sync.dma_start(out=sw_hn[:], in_=w_hn[:])
            sb_b = sb.tile([128, 4], f32)
            nc.sync.dma_start(out=sb_b[:], in_=b_all[:])
            sw_out = sb.tile([128, KT, VS], f32)
            nc.sync.dma_start(out=sw_out[:], in_=w_out[:])
            sb_vb = sb.tile([64, 1], f32)
            nc.sync.dma_start(out=sb_vb[:], in_=vbase[:])
            sb_brep = sb.tile([64, VS], bf16)
            nc.sync.dma_start(out=sb_brep[:], in_=b_rep[:])

            ident = sb.tile([128, 128], f32)
            make_identity(nc, ident[:])

            # h0 goes into the loop-carried slots directly
            sb_h0 = sbw.tile([128, KT, 64], f32, name="hg_sb", tag="hg_sb")
            nc.sync.dma_start(out=sb_h0[:], in_=h0T[:])
            sb_h0own = sbw.tile([128, 64], f32, name="new_own", tag="new_own")
            nc.sync.dma_start(out=sb_h0own[:], in_=h0own[:])

            tok0 = sb.tile([64, 1], i32)
            nc.vector.memset(tok0[:], 0)

            cur_tok = tok0          # [64,1] i32 token for this step
            cur_hg = sb_h0          # [128,KT,64] full h^T
            cur_own = sb_h0own      # [128,64] own h^T slice

            for t in range(T):
                # ---- x path: gather own hidden-slice of relu(emb[tok]) -----
                xg64 = sbw.tile([64, HS], f32, name="xg64", bufs=1)
                nc.gpsimd.indirect_dma_start(
                    out=xg64[:],
                    out_offset=None,
                    in_=embT[:],
                    in_offset=bass.IndirectOffsetOnAxis(ap=cur_tok[:, :1], axis=0),
                )
                xr = sbw.tile([64, HS], f32, name="xr", bufs=1)
                nc.scalar.activation(out=xr[:], in_=xg64[:], func=AF.Relu)
                pxT = psT.tile([128, 64], f32, name="pxT", tag="psT")
                nc.tensor.transpose(out=pxT[:], in_=xr[:], identity=ident[:64, :64])
                own_xT = sbw.tile([128, 64], f32, name="own_xT", bufs=1)
                nc.vector.tensor_copy(out=own_xT[:], in_=pxT[:])

                xin = dL.tile([128, 64], f32, name="xin")
                nc.sync.dma_start(out=xin[:], in_=own_xT[:])
                xgd = dS.tile([H, 64], f32, name="xgd", addr_space="Shared")
                nc.gpsimd.collective_compute(
                    "AllGather", OP.bypass, replica_groups=RG,
                    ins=[xin[:].opt()], outs=[xgd[:].opt()],
                )
                xg_sb = sbw.tile([128, KT, 64], f32, name="xg_sb", bufs=1)
                for k in range(KT):
                    nc.sync.dma_start(
                        out=xg_sb[:, k, :], in_=xgd[128 * k:128 * (k + 1), :]
                    )

                # ---- GRU: gates^T for this core's 128 dims -----------------
                p_r = psG.tile([128, 64], f32, name="p_r")
                p_z = psG.tile([128, 64], f32, name="p_z")
                p_in = psG.tile([128, 64], f32, name="p_in")
                p_hn = psG.tile([128, 64], f32, name="p_hn")
                # h-dependent parts first (ready before the x AllGather lands)
                for k in range(KT):
                    nc.tensor.matmul(
                        out=p_r[:], lhsT=sw_rz[:, KT + k, 0:128],
                        rhs=cur_hg[:, k, :], start=(k == 0), stop=False,
                    )
                    nc.tensor.matmul(
                        out=p_z[:], lhsT=sw_rz[:, KT + k, 128:256],
                        rhs=cur_hg[:, k, :], start=(k == 0), stop=False,
                    )
                    nc.tensor.matmul(
                        out=p_hn[:], lhsT=sw_hn[:, k, :],
                        rhs=cur_hg[:, k, :], start=(k == 0), stop=(k == KT - 1),
                    )
                for k in range(KT):
                    nc.tensor.matmul(
                        out=p_r[:], lhsT=sw_rz[:, k, 0:128],
                        rhs=xg_sb[:, k, :], start=False, stop=(k == KT - 1),
                    )
                    nc.tensor.matmul(
                        out=p_z[:], lhsT=sw_rz[:, k, 128:256],
                        rhs=xg_sb[:, k, :], start=False, stop=(k == KT - 1),
                    )
                    nc.tensor.matmul(
                        out=p_in[:], lhsT=sw_in[:, k, :],
                        rhs=xg_sb[:, k, :], start=(k == 0), stop=(k == KT - 1),
                    )

                # r = sigmoid(p_r + br) via exp: 1/(1+exp(-p_r - br))
                er = sbw.tile([128, 64], f32, name="er", bufs=1)
                nc.scalar.activation(out=er[:], in_=p_r[:], func=AF.Exp,
                                     bias=sb_b[:, 0:1], scale=-1.0)
                nc.vector.tensor_scalar_add(out=er[:], in0=er[:], scalar1=1.0)
                r_g = sbw.tile([128, 64], f32, name="r_g", bufs=1)
                nc.vector.reciprocal(out=r_g[:], in_=er[:])

                ez = sbw.tile([128, 64], f32, name="ez", bufs=1)
                nc.scalar.activation(out=ez[:], in_=p_z[:], func=AF.Exp,
                                     bias=sb_b[:, 1:2], scale=-1.0)
                nc.vector.tensor_scalar_add(out=ez[:], in0=ez[:], scalar1=1.0)
                z_g = sbw.tile([128, 64], f32, name="z_g", bufs=1)
                nc.vector.reciprocal(out=z_g[:], in_=ez[:])

                # n = tanh(p_in + b_in + r*(p_hn + b_hn)) via exp(2x)
                hnb = sbw.tile([128, 64], f32, name="hnb", bufs=1)
                nc.scalar.activation(out=hnb[:], in_=p_hn[:], func=AF.Identity,
                                     bias=sb_b[:, 3:4])
                rhn = sbw.tile([128, 64], f32, name="rhn", bufs=1)
                nc.vector.tensor_mul(out=rhn[:], in0=r_g[:], in1=hnb[:])
                nc.vector.tensor_add(out=rhn[:], in0=rhn[:], in1=p_in[:])
                en = sbw.tile([128, 64], f32, name="en", bufs=1)
                nc.scalar.activation(out=en[:], in_=rhn[:], func=AF.Exp,
                                     bias=sb_b[:, 2:3], scale=2.0)
                nc.vector.tensor_scalar_add(out=en[:], in0=en[:], scalar1=1.0)
                n_g = sbw.tile([128, 64], f32, name="n_g", bufs=1)
                nc.vector.reciprocal(out=n_g[:], in_=en[:])
                nc.vector.tensor_scalar(out=n_g[:], in0=n_g[:], scalar1=-2.0,
                                        scalar2=1.0, op0=OP.mult, op1=OP.add)

                # h' = n + z*(h - n)
                new_own = sbw.tile([128, 64], f32, name="new_own", tag="new_own")
                nc.vector.tensor_sub(out=new_own[:], in0=cur_own[:], in1=n_g[:])
                nc.vector.tensor_mul(out=new_own[:], in0=z_g[:], in1=new_own[:])
                nc.vector.tensor_add(out=new_own[:], in0=n_g[:], in1=new_own[:])

                hin = dL.tile([128, 64], f32, name="hin")
                nc.sync.dma_start(out=hin[:], in_=new_own[:])
                hgd = dS.tile([H, 64], f32, name="hgd", addr_space="Shared")
                nc.gpsimd.collective_compute(
                    "AllGather", OP.bypass, replica_groups=RG,
                    ins=[hin[:].opt()], outs=[hgd[:].opt()],
                )
                hg_sb = sbw.tile([128, KT, 64], f32, name="hg_sb", tag="hg_sb")
                for k in range(KT):
                    nc.sync.dma_start(
                        out=hg_sb[:, k, :], in_=hgd[128 * k:128 * (k + 1), :]
                    )

                # ---- logits: [64, VS] = h'^T.T @ w_outT + b_out ------------
                logits = sbL.tile([64, VS], f32, name="logits")
                for j in range(NT):
                    cs = slice(j * NW, (j + 1) * NW)
                    p_l = psL.tile([64, NW], f32, name="p_l")
                    for k in range(KT):
                        nc.tensor.matmul(
                            out=p_l[:], lhsT=hg_sb[:, k, :], rhs=sw_out[:, k, cs],
                            start=(k == 0), stop=(k == KT - 1),
                        )
                    nc.vector.tensor_tensor(out=logits[:, cs], in0=p_l[:],
                                            in1=sb_brep[:, cs], op=OP.add)

                # ---- local stats: top value + index, sum(exp(l - max)) -----
                m8 = sbw.tile([64, 8], f32, name="m8")
                nc.vector.max(out=m8[:], in_=logits[:])
                i8 = sbw.tile([64, 8], u32, name="i8")
                nc.vector.max_index(out=i8[:], in_max=m8[:], in_values=logits[:])
                negm = sbw.tile([64, 1], f32, name="negm")
                nc.vector.tensor_scalar_mul(out=negm[:], in0=m8[:, 0:1], scalar1=-1.0)
                ssum8 = sbw.tile([64, NT], f32, name="ssum8")
                for j in range(NT):
                    cs = slice(j * NW, (j + 1) * NW)
                    esc = stg.tile([64, NW], f32, name="esc")
                    nc.scalar.activation(out=esc[:], in_=logits[:, cs], func=AF.Exp,
                                         bias=negm[:, 0:1],
                                         accum_out=ssum8[:, j:j + 1])
                ssum = sbw.tile([64, 1], f32, name="ssum")
                nc.vector.tensor_reduce(out=ssum[:], in_=ssum8[:],
                                        axis=mybir.AxisListType.X, op=OP.add)
                gidx = sbw.tile([64, 1], f32, name="gidx")
                nc.vector.tensor_copy(out=gidx[:], in_=i8[:, 0:1])
                nc.vector.tensor_scalar(out=gidx[:], in0=gidx[:], scalar1=sb_vb[:, 0:1],
                                        scalar2=None, op0=OP.add)

                # pack [max, idx, sumexp] -> [3, 64] and AllGather
                cst = sbw.tile([64, 3], f32, name="cst")
                nc.vector.tensor_copy(out=cst[:, 0:1], in_=m8[:, 0:1])
                nc.vector.tensor_copy(out=cst[:, 1:2], in_=gidx[:])
                nc.vector.tensor_copy(out=cst[:, 2:3], in_=ssum[:])
                ps3 = psT.tile([3, 64], f32, name="ps3", tag="psT")
                nc.tensor.transpose(out=ps3[:], in_=cst[:], identity=ident[:64, :64])
                s3 = sbw.tile([3, 64], f32, name="s3")
                nc.vector.tensor_copy(out=s3[:], in_=ps3[:])
                sin = dL.tile([3, 64], f32, name="sin")
                nc.sync.dma_start(out=sin[:], in_=s3[:])
                sgd = dS.tile([3 * NC, 64], f32, name="sgd", addr_space="Shared")
                nc.gpsimd.collective_compute(
                    "AllGather", OP.bypass, replica_groups=RG,
                    ins=[sin[:].opt()], outs=[sgd[:].opt()],
                )
                sg_sb = sbw.tile([3 * NC, 64], f32, name="sg_sb")
                nc.sync.dma_start(out=sg_sb[:], in_=sgd[:])
                ps24 = psT.tile([64, 3 * NC], f32, name="ps24", tag="psT")
                nc.tensor.transpose(out=ps24[:], in_=sg_sb[:],
                                    identity=ident[:24, :24])
                cg = sbw.tile([64, NC, 3], f32, name="cg")
                nc.vector.tensor_copy(out=cg[:].opt(), in_=ps24[:])

                v_ap = cg[:, :, 0]
                i_ap = cg[:, :, 1]
                s_ap = cg[:, :, 2]

                gv = sbw.tile([64, 1], f32, name="gv")
                nc.vector.tensor_reduce(out=gv[:], in_=v_ap,
                                        axis=mybir.AxisListType.X, op=OP.max)
                eq = sbw.tile([64, NC], f32, name="eq")
                nc.vector.tensor_scalar(out=eq[:], in0=v_ap, scalar1=gv[:, 0:1],
                                        scalar2=None, op0=OP.is_equal)
                sel = sbw.tile([64, NC], f32, name="sel")
                nc.vector.tensor_tensor(out=sel[:], in0=eq[:], in1=i_ap, op=OP.mult)
                nc.vector.tensor_scalar(out=eq[:], in0=eq[:], scalar1=-BIG,
                                        scalar2=BIG, op0=OP.mult, op1=OP.add)
                nc.vector.tensor_add(out=sel[:], in0=sel[:], in1=eq[:])
                tokf = sbw.tile([64, 1], f32, name="tokf")
                nc.vector.tensor_reduce(out=tokf[:], in_=sel[:],
                                        axis=mybir.AxisListType.X, op=OP.min)
                ntok = sbw.tile([64, 1], i32, name="ntok")
                nc.vector.tensor_copy(out=ntok[:], in_=tokf[:])

                # LSE = gv + ln(sum_c exp(v_c - gv) * S_c)
                neggv = sbw.tile([64, 1], f32, name="neggv")
                nc.vector.tensor_scalar_mul(out=neggv[:], in0=gv[:], scalar1=-1.0)
                e8 = sbw.tile([64, NC], f32, name="e8")
                nc.scalar.activation(out=e8[:], in_=v_ap, func=AF.Exp,
                                     bias=neggv[:, 0:1])
                nc.vector.tensor_tensor(out=e8[:], in0=e8[:], in1=s_ap, op=OP.mult)
                num = sbw.tile([64, 1], f32, name="num")
                nc.vector.tensor_reduce(out=num[:], in_=e8[:],
                                        axis=mybir.AxisListType.X, op=OP.add)
                nlse = sbw.tile([64, 1], f32, name="nlse")
                nc.scalar.activation(out=nlse[:], in_=num[:], func=AF.Ln)
                nc.vector.tensor_add(out=nlse[:], in0=nlse[:], in1=gv[:])
                nc.vector.tensor_scalar_mul(out=nlse[:], in0=nlse[:], scalar1=-1.0)

                # ---- output: log_probs[t] = logits - LSE -------------------
                for j in range(NT):
                    cs = slice(j * NW, (j + 1) * NW)
                    og = stg.tile([64, NW], f32, name="og")
                    nc.scalar.activation(out=og[:], in_=logits[:, cs],
                                         func=AF.Identity, bias=nlse[:, 0:1])
                    nc.sync.dma_start(out=lp[t][:, cs], in_=og[:])

                cur_tok = ntok
                cur_hg = hg_sb
                cur_own = new_own

            nc.sync.dma_start(out=hfin[:], in_=cur_own[:])

    return nc


@functools.lru_cache(maxsize=1)
def _get_nc():
    nc = _build_nc()
    nc.finalize()
    return nc


def _prep_in_maps(encoder_hidden, embedding, w_ih, w_hh, b_ih, b_hh, w_out, b_out):
    f = np.float32
    h0 = np.asarray(encoder_hidden, f)[0]              # [B, H]
    emb = np.asarray(embedding, f)
    w_ih = np.asarray(w_ih, f)
    w_hh = np.asarray(w_hh, f)
    b_ih = np.asarray(b_ih, f)
    b_hh = np.asarray(b_hh, f)
    w_out = np.asarray(w_out, f)
    b_out = np.asarray(b_out, f)

    h0T_full = np.ascontiguousarray(h0.T)              # [H, B]
    h0T_tiled = np.ascontiguousarray(
        h0T_full.reshape(KT, 128, B).transpose(1, 0, 2))

    in_maps = []
    for c in range(NC):
        r_rows = np.arange(c * 128, (c + 1) * 128)
        z_rows = H + r_rows
        n_rows = 2 * H + r_rows

        w_rz = np.concatenate(
            [
                np.concatenate([w_ih[r_rows].T, w_ih[z_rows].T], axis=1),   # [H, 256]
                np.concatenate([w_hh[r_rows].T, w_hh[z_rows].T], axis=1),   # [H, 256]
            ],
            axis=0,
        )                                               # [2H, 256]
        w_rz = np.ascontiguousarray(
            w_rz.reshape(2 * KT, 128, 256).transpose(1, 0, 2))

        w_in = np.ascontiguousarray(
            w_ih[n_rows].T.reshape(KT, 128, 128).transpose(1, 0, 2))
        w_hn = np.ascontiguousarray(
            w_hh[n_rows].T.reshape(KT, 128, 128).transpose(1, 0, 2))

        b_allc = np.stack(
            [
                -(b_ih[r_rows] + b_hh[r_rows]),
                -(b_ih[z_rows] + b_hh[z_rows]),
                2.0 * b_ih[n_rows],
                b_hh[n_rows],
            ],
            axis=1,
        ).astype(f)                                     # [128, 4]

        w_oc = np.ascontiguousarray(
            w_out[c * VS:(c + 1) * VS].T.reshape(KT, 128, VS).transpose(1, 0, 2))

        in_maps.append({
            "w_rz": w_rz,
            "w_in": w_in,
            "w_hn": w_hn,
            "b_all": b_allc,
            "w_out": w_oc,
            "b_rep": np.ascontiguousarray(
                np.broadcast_to(b_out[c * VS:(c + 1) * VS][None, :], (64, VS))
            ).astype(ml_dtypes.bfloat16),
            "embT": np.ascontiguousarray(emb[:, c * HS:(c + 1) * HS]),
            "h0T": h0T_tiled,
            "h0own": np.ascontiguousarray(h0T_full[c * 128:(c + 1) * 128]),
            "vbase": np.full((64, 1), c * VS, f),
        })
    return in_maps


def _assemble(results):
    lp = np.concatenate([results[c]["lp"] for c in range(NC)], axis=2)  # [T,B,V]
    log_probs = np.ascontiguousarray(lp.transpose(1, 0, 2))             # [B,T,V]
    hT = np.concatenate([results[c]["hfin"] for c in range(NC)], axis=0)  # [H,B]
    h_final = np.ascontiguousarray(hT.T)[None]                          # [1,B,H]
    return log_probs, h_final


def kernel(encoder_outputs, encoder_hidden, embedding, w_ih, w_hh,
           b_ih, b_hh, w_out, b_out):
    from concourse.bass_utils import run_bass_kernel_spmd

    in_maps = _prep_in_maps(encoder_hidden, embedding, w_ih, w_hh,
                            b_ih, b_hh, w_out, b_out)
    res = run_bass_kernel_spmd(_get_nc(), in_maps, core_ids=list(range(NC)))
    return _assemble(res.results)


# revision 11
# speedup vs baseline: 721.1455x; 721.1455x over previous
"""Trainium2 Bass kernel for a GRU decoder with greedy argmax feedback.

Model (13 serial steps, B=64, H=1024, V=32000):
    x      = relu(embedding[tok])
    h      = GRU(x, h)                      # PyTorch gate order r,z,n
    logits = h @ w_out.T + b_out            # [B, V]
    tok    = argmax(logits)                 # feeds next step
Output: log_softmax(logits over all steps)  [B, T, V], and final h.

Sharding over 8 NeuronCores (one TRN2 chip, LNC1):
  - w_out + output logits: vocab-sharded (4000 cols/core, weights resident in SBUF)
  - GRU: gate-dim sharded (each core computes 128 of each of r/z/n) -> AllGather h^T
  - embedding: hidden-dim sharded (each core gathers its 128-wide slice) -> AllGather x^T
  - argmax + log-sum-exp: per-core stats AllGathered, combined identically on all cores

Per step: 3 tiny AllGathers (x^T slices, h^T slices, [max, argmax-idx, sumexp] stats).
All matmuls fp32: the argmax feedback chain cannot tolerate bf16 noise (one flipped
token diverges the whole remaining sequence).
"""

import functools

import ml_dtypes
import numpy as np

B = 64
H = 1024
V = 32000
T = 13
NC = 8
VS = V // NC        # 4000 vocab cols per core
HS = H // NC        # 128 hidden dims per core
KT = H // 128       # 8 K-tiles over hidden
NT = 8              # logits N-tiles
NW = VS // NT       # 500 cols per N-tile (<=512 fp32 moving limit)
PAD = 0
BIG = 1.0e9

RG = [list(range(NC))]


def _build_nc():
    from concourse import bacc, mybir
    import concourse.tile as tile
    from concourse.masks import make_identity
    import concourse.bass as bass

    f32 = mybir.dt.float32
    bf16 = mybir.dt.bfloat16
    i32 = mybir.dt.int32
    u32 = mybir.dt.uint32
    AF = mybir.ActivationFunctionType
    OP = mybir.AluOpType

    nc = bacc.Bacc("TRN2", target_bir_lowering=False, debug=False, num_devices=NC)

    def inp(name, shape, dtype=f32):
        return nc.dram_tensor(name, shape, dtype, kind="ExternalInput")

    def outp(name, shape, dtype=f32):
        return nc.dram_tensor(name, shape, dtype, kind="ExternalOutput")

    w_rz = inp("w_rz", [128, 2 * KT, 256])      # [x|h] K-tiles, cols [r|z]
    w_in = inp("w_in", [128, KT, 128])          # w_ih n-gate slice (transposed)
    w_hn = inp("w_hn", [128, KT, 128])          # w_hh n-gate slice (transposed)
    b_all = inp("b_all", [128, 4])              # cols: -(br), -(bz), 2*b_ih_n, b_hh_n
    w_out = inp("w_out", [128, KT, VS])         # w_out slice, transposed, K-tiled
    b_rep = inp("b_rep", [64, VS], bf16)        # b_out slice replicated over batch
    embT = inp("embT", [V, HS])                 # embedding hidden-slice
    h0T = inp("h0T", [128, KT, 64])             # full h0^T, K-tiled
    h0own = inp("h0own", [128, 64])             # this core's h0^T slice
    vbase = inp("vbase", [64, 1])               # this core's vocab base (c*VS) as f32
    lp = outp("lp", [T, B, VS])
    hfin = outp("hfin", [128, 64])

    with tile.TileContext(nc) as tc:
        with (
            tc.tile_pool(name="sb", bufs=1) as sb,
            tc.tile_pool(name="sbw", bufs=2) as sbw,
            tc.tile_pool(name="sbL", bufs=1) as sbL,
            tc.tile_pool(name="stg", bufs=2) as stg,
            tc.tile_pool(name="psG", bufs=1, space="PSUM") as psG,
            tc.tile_pool(name="psL", bufs=2, space="PSUM") as psL,
            tc.tile_pool(name="psT", bufs=2, space="PSUM") as psT,
            tc.tile_pool(name="dL", bufs=2, space="DRAM") as dL,
            tc.tile_pool(name="dS", bufs=2, space="DRAM") as dS,
        ):
            # ---- static setup ----------------------------------------------
            sw_rz = sb.tile([128, 2 * KT, 256], f32)
            nc.sync.dma_start(out=sw_rz[:], in_=w_rz[:])
            sw_in = sb.tile([128, KT, 128], f32)
            nc.sync.dma_start(out=sw_in[:], in_=w_in[:])
            sw_hn = sb.tile([128, KT, 128], f32)
            nc.sync.dma_start(out=sw_hn[:], in_=w_hn[:])
            sb_b = sb.tile([128, 4], f32)
            nc.sync.dma_start(out=sb_b[:], in_=b_all[:])
            sw_out = sb.tile([128, KT, VS], f32)
            nc.sync.dma_start(out=sw_out[:], in_=w_out[:])
            sb_vb = sb.tile([64, 1], f32)
            nc.sync.dma_start(out=sb_vb[:], in_=vbase[:])
            sb_brep = sb.tile([64, VS], bf16)
            nc.sync.dma_start(out=sb_brep[:], in_=b_rep[:])

            ident = sb.tile([128, 128], f32)
            make_identity(nc, ident[:])

            # h0 goes into the loop-carried slots directly
            sb_h0 = sbw.tile([128, KT, 64], f32, name="hg_sb", tag="hg_sb")
            nc.sync.dma_start(out=sb_h0[:], in_=h0T[:])
            sb_h0own = sbw.tile([128, 64], f32, name="new_own", tag="new_own")
            nc.sync.dma_start(out=sb_h0own[:], in_=h0own[:])

            tok0 = sb.tile([64, 1], i32)
            nc.vector.memset(tok0[:], 0)

            cur_tok = tok0          # [64,1] i32 token for this step
            cur_hg = sb_h0          # [128,KT,64] full h^T
            cur_own = sb_h0own      # [128,64] own h^T slice

            for t in range(T):
                # ---- x path: gather own hidden-slice of relu(emb[tok]) -----
                xg64 = sbw.tile([64, HS], f32, name="xg64", bufs=1)
                nc.gpsimd.indirect_dma_start(
                    out=xg64[:],
                    out_offset=None,
                    in_=embT[:],
                    in_offset=bass.IndirectOffsetOnAxis(ap=cur_tok[:, :1], axis=0),
                )
                xr = sbw.tile([64, HS], f32, name="xr", bufs=1)
                nc.scalar.activation(out=xr[:], in_=xg64[:], func=AF.Relu)
                pxT = psT.tile([128, 64], f32, name="pxT", tag="psT")
                nc.tensor.transpose(out=pxT[:], in_=xr[:], identity=ident[:64, :64])
                own_xT = sbw.tile([128, 64], f32, name="own_xT", bufs=1)
                nc.vector.tensor_copy(out=own_xT[:], in_=pxT[:])

                xin = dL.tile([128, 64], f32, name="xin")
                nc.sync.dma_start(out=xin[:], in_=own_xT[:])
                xgd = dS.tile([H, 64], f32, name="xgd", addr_space="Shared")
                nc.gpsimd.collective_compute(
                    "AllGather", OP.bypass, replica_groups=RG,
                    ins=[xin[:].opt()], outs=[xgd[:].opt()],
                )
                xg_sb = sbw.tile([128, KT, 64], f32, name="xg_sb", bufs=1)
                for k in range(KT):
                    nc.sync.dma_start(
                        out=xg_sb[:, k, :], in_=xgd[128 * k:128 * (k + 1), :]
                    )

                # ---- GRU: gates^T for this core's 128 dims -----------------
                p_r = psG.tile([128, 64], f32, name="p_r")
                p_z = psG.tile([128, 64], f32, name="p_z")
                p_in = psG.tile([128, 64], f32, name="p_in")
                p_hn = psG.tile([128, 64], f32, name="p_hn")
                # h-dependent parts first (ready before the x AllGather lands)
                for k in range(KT):
                    nc.tensor.matmul(
                        out=p_r[:], lhsT=sw_rz[:, KT + k, 0:128],
                        rhs=cur_hg[:, k, :], start=(k == 0), stop=False,
                    )
                    nc.tensor.matmul(
                        out=p_z[:], lhsT=sw_rz[:, KT + k, 128:256],
                        rhs=cur_hg[:, k, :], start=(k == 0), stop=False,
                    )
                    nc.tensor.matmul(
                        out=p_hn[:], lhsT=sw_hn[:, k, :],
                        rhs=cur_hg[:, k, :], start=(k == 0), stop=(k == KT - 1),
                    )
                for k in range(KT):
                    nc.tensor.matmul(
                        out=p_r[:], lhsT=sw_rz[:, k, 0:128],
                        rhs=xg_sb[:, k, :], start=False, stop=(k == KT - 1),
                    )
                    nc.tensor.matmul(
                        out=p_z[:], lhsT=sw_rz[:, k, 128:256],
                        rhs=xg_sb[:, k, :], start=False, stop=(k == KT - 1),
                    )
                    nc.tensor.matmul(
                        out=p_in[:], lhsT=sw_in[:, k, :],
                        rhs=xg_sb[:, k, :], start=(k == 0), stop=(k == KT - 1),
                    )

                # r = sigmoid(p_r + br) via exp: 1/(1+exp(-p_r - br))
                er = sbw.tile([128, 64], f32, name="er", bufs=1)
                nc.scalar.activation(out=er[:], in_=p_r[:], func=AF.Exp,
                                     bias=sb_b[:, 0:1], scale=-1.0)
                nc.vector.tensor_scalar_add(out=er[:], in0=er[:], scalar1=1.0)
                r_g = sbw.tile([128, 64], f32, name="r_g", bufs=1)
                nc.vector.reciprocal(out=r_g[:], in_=er[:])

                ez = sbw.tile([128, 64], f32, name="ez", bufs=1)
                nc.scalar.activation(out=ez[:], in_=p_z[:], func=AF.Exp,
                                     bias=sb_b[:, 1:2], scale=-1.0)
                nc.vector.tensor_scalar_add(out=ez[:], in0=ez[:], scalar1=1.0)
                z_g = sbw.tile([128, 64], f32, name="z_g", bufs=1)
                nc.vector.reciprocal(out=z_g[:], in_=ez[:])

                # n = tanh(p_in + b_in + r*(p_hn + b_hn)) via exp(2x)
                hnb = sbw.tile([128, 64], f32, name="hnb", bufs=1)
                nc.scalar.activation(out=hnb[:], in_=p_hn[:], func=AF.Identity,
                                     bias=sb_b[:, 3:4])
                rhn = sbw.tile([128, 64], f32, name="rhn", bufs=1)
                nc.vector.tensor_mul(out=rhn[:], in0=r_g[:], in1=hnb[:])
                nc.vector.tensor_add(out=rhn[:], in0=rhn[:], in1=p_in[:])
                en = sbw.tile([128, 64], f32, name="en", bufs=1)
                nc.scalar.activation(out=en[:], in_=rhn[:], func=AF.Exp,
                                     bias=sb_b[:, 2:3], scale=2.0)
                nc.vector.tensor_scalar_add(out=en[:], in0=en[:], scalar1=1.0)
                n_g = sbw.tile([128, 64], f32, name="n_g", bufs=1)
                nc.vector.reciprocal(out=n_g[:], in_=en[:])
                nc.vector.tensor_scalar(out=n_g[:], in0=n_g[:], scalar1=-2.0,
                                        scalar2=1.0, op0=OP.mult, op1=OP.add)

                # h' = n + z*(h - n)
                new_own = sbw.tile([128, 64], f32, name="new_own", tag="new_own")
                nc.vector.tensor_sub(out=new_own[:], in0=cur_own[:], in1=n_g[:])
                nc.vector.tensor_mul(out=new_own[:], in0=z_g[:], in1=new_own[:])
                nc.vector.tensor_add(out=new_own[:], in0=n_g[:], in1=new_own[:])

                hin = dL.tile([128, 64], f32, name="hin")
                nc.sync.dma_start(out=hin[:], in_=new_own[:])
                hgd = dS.tile([H, 64], f32, name="hgd", addr_space="Shared")
                nc.gpsimd.collective_compute(
                    "AllGather", OP.bypass, replica_groups=RG,
                    ins=[hin[:].opt()], outs=[hgd[:].opt()],
                )
                hg_sb = sbw.tile([128, KT, 64], f32, name="hg_sb", tag="hg_sb")
                for k in range(KT):
                    nc.sync.dma_start(
                        out=hg_sb[:, k, :], in_=hgd[128 * k:128 * (k + 1), :]
                    )

                # ---- logits: [64, VS] = h'^T.T @ w_outT + b_out ------------
                logits = sbL.tile([64, VS], f32, name="logits")
                for j in range(NT):
                    cs = slice(j * NW, (j + 1) * NW)
                    p_l = psL.tile([64, NW], f32, name="p_l")
                    for k in range(KT):
                        nc.tensor.matmul(
                            out=p_l[:], lhsT=hg_sb[:, k, :], rhs=sw_out[:, k, cs],
                            start=(k == 0), stop=(k == KT - 1),
                        )
                    nc.vector.tensor_tensor(out=logits[:, cs], in0=p_l[:],
                                            in1=sb_brep[:, cs], op=OP.add)

                # ---- local stats: top value + index, sum(exp(l - max)) -----
                m8 = sbw.tile([64, 8], f32, name="m8")
                nc.vector.max(out=m8[:], in_=logits[:])
                i8 = sbw.tile([64, 8], u32, name="i8")
                nc.vector.max_index(out=i8[:], in_max=m8[:], in_values=logits[:])
                negm = sbw.tile([64, 1], f32, name="negm")
                nc.vector.tensor_scalar_mul(out=negm[:], in0=m8[:, 0:1], scalar1=-1.0)
                ssum8 = sbw.tile([64, NT], f32, name="ssum8")
                for j in range(NT):
                    cs = slice(j * NW, (j + 1) * NW)
                    esc = stg.tile([64, NW], f32, name="esc")
                    nc.scalar.activation(out=esc[:], in_=logits[:, cs], func=AF.Exp,
                                         bias=negm[:, 0:1],
                                         accum_out=ssum8[:, j:j + 1])
                ssum = sbw.tile([64, 1], f32, name="ssum")
                nc.vector.tensor_reduce(out=ssum[:], in_=ssum8[:],
                                        axis=mybir.AxisListType.X, op=OP.add)
                gidx = sbw.tile([64, 1], f32, name="gidx")
                nc.vector.tensor_copy(out=gidx[:], in_=i8[:, 0:1])
                nc.vector.tensor_scalar(out=gidx[:], in0=gidx[:], scalar1=sb_vb[:, 0:1],
                                        scalar2=None, op0=OP.add)

                # pack [max, idx, sumexp] -> [3, 64] and AllGather
                cst = sbw.tile([64, 3], f32, name="cst")
                nc.vector.tensor_copy(out=cst[:, 0:1], in_=m8[:, 0:1])
                nc.vector.tensor_copy(out=cst[:, 1:2], in_=gidx[:])
                nc.vector.tensor_copy(out=cst[:, 2:3], in_=ssum[:])
                ps3 = psT.tile([3, 64], f32, name="ps3", tag="psT")
                nc.tensor.transpose(out=ps3[:], in_=cst[:], identity=ident[:64, :64])
                s3 = sbw.tile([3, 64], f32, name="s3")
                nc.vector.tensor_copy(out=s3[:], in_=ps3[:])
                sin = dL.tile([3, 64], f32, name="sin")
                nc.sync.dma_start(out=sin[:], in_=s3[:])
                sgd = dS.tile([3 * NC, 64], f32, name="sgd", addr_space="Shared")
                nc.gpsimd.collective_compute(
                    "AllGather", OP.bypass, replica_groups=RG,
                    ins=[sin[:].opt()], outs=[sgd[:].opt()],
                )
                sg_sb = sbw.tile([3 * NC, 64], f32, name="sg_sb")
                nc.sync.dma_start(out=sg_sb[:], in_=sgd[:])
                ps24 = psT.tile([64, 3 * NC], f32, name="ps24", tag="psT")
                nc.tensor.transpose(out=ps24[:], in_=sg_sb[:],
                                    identity=ident[:24, :24])
                cg = sbw.tile([64, NC, 3], f32, name="cg")
                nc.vector.tensor_copy(out=cg[:].opt(), in_=ps24[:])

                v_ap = cg[:, :, 0]
                i_ap = cg[:, :, 1]
                s_ap = cg[:, :, 2]

                gv = sbw.tile([64, 1], f32, name="gv")
                nc.vector.tensor_reduce(out=gv[:], in_=v_ap,
                                        axis=mybir.AxisListType.X, op=OP.max)
                eq = sbw.tile([64, NC], f32, name="eq")
                nc.vector.tensor_scalar(out=eq[:], in0=v_ap, scalar1=gv[:, 0:1],
                                        scalar2=None, op0=OP.is_equal)
                sel = sbw.tile([64, NC], f32, name="sel")
                nc.vector.tensor_tensor(out=sel[:], in0=eq[:], in1=i_ap, op=OP.mult)
                nc.vector.tensor_scalar(out=eq[:], in0=eq[:], scalar1=-BIG,
                                        scalar2=BIG, op0=OP.mult, op1=OP.add)
                nc.vector.tensor_add(out=sel[:], in0=sel[:], in1=eq[:])
                tokf = sbw.tile([64, 1], f32, name="tokf")
                nc.vector.tensor_reduce(out=tokf[:], in_=sel[:],
                                        axis=mybir.AxisListType.X, op=OP.min)
                ntok = sbw.tile([64, 1], i32, name="ntok")
                nc.vector.tensor_copy(out=ntok[:], in_=tokf[:])

                # LSE = gv + ln(sum_c exp(v_c - gv) * S_c)
                neggv = sbw.tile([64, 1], f32, name="neggv")
                nc.vector.tensor_scalar_mul(out=neggv[:], in0=gv[:], scalar1=-1.0)
                e8 = sbw.tile([64, NC], f32, name="e8")
                nc.scalar.activation(out=e8[:], in_=v_ap, func=AF.Exp,
                                     bias=neggv[:, 0:1])
                nc.vector.tensor_tensor(out=e8[:], in0=e8[:], in1=s_ap, op=OP.mult)
                num = sbw.tile([64, 1], f32, name="num")
                nc.vector.tensor_reduce(out=num[:], in_=e8[:],
                                        axis=mybir.AxisListType.X, op=OP.add)
                nlse = sbw.tile([64, 1], f32, name="nlse")
                nc.scalar.activation(out=nlse[:], in_=num[:], func=AF.Ln)
                nc.vector.tensor_add(out=nlse[:], in0=nlse[:], in1=gv[:])
                nc.vector.tensor_scalar_mul(out=nlse[:], in0=nlse[:], scalar1=-1.0)

                # ---- output: log_probs[t] = logits - LSE -------------------
                for j in range(NT):
                    cs = slice(j * NW, (j + 1) * NW)
                    og = stg.tile([64, NW], f32, name="og")
                    nc.scalar.activation(out=og[:], in_=logits[:, cs],
                                         func=AF.Identity, bias=nlse[:, 0:1])
                    nc.sync.dma_start(out=lp[t][:, cs], in_=og[:])

                cur_tok = ntok
                cur_hg = hg_sb
                cur_own = new_own

            nc.sync.dma_start(out=hfin[:], in_=cur_own[:])

    return nc


@functools.lru_cache(maxsize=1)
def _get_nc():
    nc = _build_nc()
    nc.finalize()
    return nc


def _prep_in_maps(encoder_hidden, embedding, w_ih, w_hh, b_ih, b_hh, w_out, b_out):
    f = np.float32
    h0 = np.asarray(encoder_hidden, f)[0]              # [B, H]
    emb = np.asarray(embedding, f)
    w_ih = np.asarray(w_ih, f)
    w_hh = np.asarray(w_hh, f)
    b_ih = np.asarray(b_ih, f)
    b_hh = np.asarray(b_hh, f)
    w_out = np.asarray(w_out, f)
    b_out = np.asarray(b_out, f)

    h0T_full = np.ascontiguousarray(h0.T)              # [H, B]
    h0T_tiled = np.ascontiguousarray(
        h0T_full.reshape(KT, 128, B).transpose(1, 0, 2))

    in_maps = []
    for c in range(NC):
        r_rows = np.arange(c * 128, (c + 1) * 128)
        z_rows = H + r_rows
        n_rows = 2 * H + r_rows

        w_rz = np.concatenate(
            [
                np.concatenate([w_ih[r_rows].T, w_ih[z_rows].T], axis=1),   # [H, 256]
                np.concatenate([w_hh[r_rows].T, w_hh[z_rows].T], axis=1),   # [H, 256]
            ],
            axis=0,
        )                                               # [2H, 256]
        w_rz = np.ascontiguousarray(
            w_rz.reshape(2 * KT, 128, 256).transpose(1, 0, 2))

        w_in = np.ascontiguousarray(
            w_ih[n_rows].T.reshape(KT, 128, 128).transpose(1, 0, 2))
        w_hn = np.ascontiguousarray(
            w_hh[n_rows].T.reshape(KT, 128, 128).transpose(1, 0, 2))

        b_allc = np.stack(
            [
                -(b_ih[r_rows] + b_hh[r_rows]),
                -(b_ih[z_rows] + b_hh[z_rows]),
                2.0 * b_ih[n_rows],
                b_hh[n_rows],
            ],
            axis=1,
        ).astype(f)                                     # [128, 4]

        w_oc = np.ascontiguousarray(
            w_out[c * VS:(c + 1) * VS].T.reshape(KT, 128, VS).transpose(1, 0, 2))

        in_maps.append({
            "w_rz": w_rz,
            "w_in": w_in,
            "w_hn": w_hn,
            "b_all": b_allc,
            "w_out": w_oc,
            "b_rep": np.ascontiguousarray(
                np.broadcast_to(b_out[c * VS:(c + 1) * VS][None, :], (64, VS))
            ).astype(ml_dtypes.bfloat16),
            "embT": np.ascontiguousarray(emb[:, c * HS:(c + 1) * HS]),
            "h0T": h0T_tiled,
            "h0own": np.ascontiguousarray(h0T_full[c * 128:(c + 1) * 128]),
            "vbase": np.full((64, 1), c * VS, f),
        })
    return in_maps


def _assemble(results):
    lp = np.concatenate([results[c]["lp"] for c in range(NC)], axis=2)  # [T,B,V]
    log_probs = np.ascontiguousarray(lp.transpose(1, 0, 2))             # [B,T,V]
    hT = np.concatenate([results[c]["hfin"] for c in range(NC)], axis=0)  # [H,B]
    h_final = np.ascontiguousarray(hT.T)[None]                          # [1,B,H]
    return log_probs, h_final


@functools.lru_cache(maxsize=1)
def _get_runner():
    """Compile once; return (sharded_fn, in_names, out_names, out_avals).

    Mirrors concourse.bass2jax.run_bass_via_pjrt but caches the jitted
    executable so repeat kernel() calls skip re-trace/re-compile.
    """
    import jax
    from jax.experimental.shard_map import shard_map
    from jax.sharding import Mesh, PartitionSpec
    from concourse import bass2jax, mybir

    bass2jax.install_neuronx_cc_hook()
    nc = _get_nc()
    assert nc.dbg_addr is None
    partition_name = (nc.partition_id_tensor.name
                      if nc.partition_id_tensor else None)

    in_names, out_names, out_avals = [], [], []
    for alloc in nc.m.functions[0].allocations:
        if not isinstance(alloc, mybir.MemoryLocationSet):
            continue
        name = alloc.memorylocations[0].name
        if alloc.kind == "ExternalInput":
            if name != partition_name:
                in_names.append(name)
        elif alloc.kind == "ExternalOutput":
            shape = tuple(alloc.tensor_shape)
            out_names.append(name)
            out_avals.append(
                jax.core.ShapedArray(shape, mybir.dt.np(alloc.dtype)))
    n_params = len(in_names)
    all_names = in_names + out_names
    if partition_name is not None:
        all_names = all_names + [partition_name]

    def _body(*args):
        operands = list(args)
        if partition_name is not None:
            operands.append(bass2jax.partition_id_tensor())
        return tuple(bass2jax._bass_exec_p.bind(
            *operands,
            out_avals=tuple(out_avals),
            in_names=tuple(all_names),
            out_names=tuple(out_names),
            lowering_input_output_aliases=(),
            sim_require_finite=True,
            sim_require_nnan=True,
            nc=nc,
        ))

    devices = jax.devices()[:NC]
    mesh = Mesh(np.asarray(devices), ("core",))
    n_out = len(out_names)
    sharded = jax.jit(
        shard_map(
            _body, mesh=mesh,
            in_specs=(PartitionSpec("core"),) * (n_params + n_out),
            out_specs=(PartitionSpec("core"),) * n_out,
            check_rep=False,
        ),
        donate_argnums=tuple(range(n_params, n_params + n_out)),
        keep_unused=True,
    )
    return sharded, in_names, out_names, out_avals


def _run(in_maps):
    sharded, in_names, out_names, out_avals = _get_runner()
    concat_in = [
        np.concatenate([np.asarray(in_maps[c][k]) for c in range(NC)], axis=0)
        for k in in_names
    ]
    zeros = [np.zeros((NC * a.shape[0], *a.shape[1:]), a.dtype)
             for a in out_avals]
    out = sharded(*concat_in, *zeros)
    return [
        {k: np.asarray(out[i]).reshape(NC, *out_avals[i].shape)[c]
         for i, k in enumerate(out_names)}
        for c in range(NC)
    ]


def kernel(encoder_outputs, encoder_hidden, embedding, w_ih, w_hh,
           b_ih, b_hh, w_out, b_out):
    in_maps = _prep_in_maps(encoder_hidden, embedding, w_ih, w_hh,
                            b_ih, b_hh, w_out, b_out)
    return _assemble(_run(in_maps))
